# revision 5
# baseline (speedup 1.0000x reference)
"""CWTConvNet Trainium2 kernel (v3: raw bacc, 2-pass mixed-precision conv).

The reference computes a 112-filter Morlet-wavelet SAME conv over length-2048
signals, then gathers output positions IMG_SELECT = linspace(0, 71, 224) cast
to int64 — only conv positions 0..71 survive. For those positions, only filter
taps k in [209, 561) touch non-pad input, so the module reduces to

    out72[f, s, l] = sum_{j} w2[f, j] * xe[s, j + l],   l in [0, 72)

with w2 = w_real[:, 0, 209:209+J] and xe = [71 zeros, x[s, :J], ...].

v3 choices (validated numerically; rel err ~6e-3 vs the 2e-2 budget):
- J truncated 352 -> 192: taps >= +121 past the wavelet center carry
  negligible energy. 2 contraction passes of 96 taps each.
- Pass-0 (taps 0..95, incl. the wavelet centers) streams the x-im2col in
  bf16; pass-1 (taps 96..191, Gaussian tails) streams it in fp8-e4m3.
  Weights stay bf16 both passes (mixed-dtype matmul). Output drained
  PSUM->SBUF as bf16. Total DMA ~1.8 MB/core vs 4.1 MB for the baseline.
- Raw bacc with manual semaphores. Both weight passes ride ONE leading DMA
  on the sync ring (tiny-descriptor weight DMAs otherwise starve behind the
  big im2col reads and gate the first matmul). im2col reads are split in
  column halves for matmul/DMA pipelining; per-bank-pair stores chase the
  drains. Dummy warm-up matmuls keep the PE HAM clock-gate busy during the
  DMA fill so the real matmuls run at 2.4 GHz.

Per core (pure data parallel over 4 of 32 batches = 48 signals): one
48-signal element-interleaved group; im2col columns c = 48*l + k so every
DMA row is a 3456 B contiguous run. Host undoes the interleave and applies
the IMG_SELECT repeat-gather on the bf16 result.
"""

import numpy as np

import concourse.bacc as bacc
import concourse.bass as bass
import concourse.mybir as mybir
from concourse.bass_utils import run_bass_kernel_spmd

# Problem constants (hardcoded; kernel.py must be self-contained).
B, C, L = 32, 12, 2048
F = 112
NCORES = 8
BPC = B // NCORES          # batches per core
S = BPC * C                # signals per core (48)
NL = 72                    # conv output positions actually used
NI = 224                   # expanded output length
KOFF = 209                 # first needed tap of the padded-filter window
J = 192                    # taps kept (truncation error ~1e-3 of output norm)
K0 = 96                    # pass-0 taps (bf16)
K1 = J - K0                # pass-1 taps (fp8 x-side), 96
XE_LEN = K0 + K1 - 1 + 71 + 1  # max t touched: 96+95+71 = 262 -> 263
NCOL = S * NL              # 3456 matmul columns
NBANK = 8                  # PSUM banks
NCOL_B = NCOL // NBANK     # 432 columns per bank
HALF = NCOL // 2           # column half for pipelining (1728)
NWARM = 54                 # HAM warm-up dummy matmuls (~53 ns each cold)

SEL = np.linspace(0, 71, NI, dtype=np.int64)

f32 = mybir.dt.float32
bf16 = mybir.dt.bfloat16
fp8 = mybir.dt.float8e4

_CACHE = {}


def _build_nc():
    nc = bacc.Bacc("TRN2", target_bir_lowering=False, debug=False)

    xgb_d = nc.declare_dram_parameter("xgb", [XE_LEN * S], bf16, isOutput=False)
    xg8_d = nc.declare_dram_parameter("xg8", [(XE_LEN - K0) * S], fp8, isOutput=False)
    w_d = nc.declare_dram_parameter("wt", [K0, 2, F], bf16, isOutput=False)
    y_d = nc.declare_dram_parameter("y", [F, NCOL], bf16, isOutput=True)

    wt = nc.alloc_sbuf_tensor("wts", [K0, 2, F], bf16)
    rhs0 = nc.alloc_sbuf_tensor("rhs0", [K0, NCOL], bf16)
    rhs1 = nc.alloc_sbuf_tensor("rhs1", [K1, NCOL], fp8)
    o = nc.alloc_sbuf_tensor("o", [F, NCOL], bf16)
    pr0 = nc.alloc_sbuf_tensor("pr0", [1, 8], bf16)
    pr1 = nc.alloc_sbuf_tensor("pr1", [1, 8], bf16)
    ps = nc.alloc_psum_tensor("ps", [128, NBANK, 512], f32)

    qs = nc.alloc_semaphore("qs")      # sync-queue DMA completions
    qa = nc.alloc_semaphore("qa")      # scalar-queue DMA completions
    msem = nc.alloc_semaphore("msem")  # per-bank matmul-group completions
    vsem = nc.alloc_semaphore("vsem")  # vector drains (even banks)
    ssem = nc.alloc_semaphore("ssem")  # scalar drains (odd banks)
    osem = nc.alloc_semaphore("osem")  # store completions

    def rhs_src(tensor, h):
        return bass.AP(tensor=tensor, offset=HALF * h, ap=[[S, K0], [1, HALF]])

    with nc.Block() as blk:

        @blk.sync
        def _(sync: bass.BassEngine):
            sync.dma_start(rhs0[:, :HALF], rhs_src(xgb_d, 0)).then_inc(qs, 16)
            sync.dma_start(rhs0[:, HALF:], rhs_src(xgb_d, 1)).then_inc(qs, 16)
            # Stores: one per bank, chasing the drains.
            for b in range(NBANK):
                sync.wait_ge(vsem if b % 2 == 0 else ssem, b // 2 + 1)
                cols = slice(b * NCOL_B, (b + 1) * NCOL_B)
                sync.dma_start(y_d.ap()[:, cols], o[:, cols]).then_inc(osem, 16)
            sync.wait_ge(osem, 128)

        @blk.scalar
        def _(scalar: bass.BassEngine):
            # Weights first: small, and they gate every matmul. On the scalar
            # ring they do not delay the big bf16 reads on the sync ring.
            scalar.dma_start(wt[:], w_d.ap()).then_inc(qa, 16)
            scalar.dma_start(rhs1[:, :HALF], rhs_src(xg8_d, 0)).then_inc(qa, 16)
            scalar.dma_start(rhs1[:, HALF:], rhs_src(xg8_d, 1)).then_inc(qa, 16)
            # Prime the ACT table load during the DMA fill phase, off the
            # drain critical path.
            scalar.copy(pr1[:], pr0[:])
            for b in (1, 3, 5, 7):
                scalar.wait_ge(msem, b + 1)
                cols = slice(b * NCOL_B, (b + 1) * NCOL_B)
                scalar.copy(o[:, cols], ps[:F, b, :NCOL_B]).then_inc(ssem, 1)

        @blk.vector
        def _(vector: bass.BassEngine):
            for b in (0, 2, 4, 6):
                vector.wait_ge(msem, b + 1)
                cols = slice(b * NCOL_B, (b + 1) * NCOL_B)
                vector.tensor_copy(
                    out=o[:, cols], in_=ps[:F, b, :NCOL_B]
                ).then_inc(vsem, 1)

        @blk.tensor
        def _(tensor: bass.BassEngine):
            # HAM warm-up on whatever bytes happen to be in SBUF; results go
            # to a PSUM region every real group later resets (start=True).
            for _i in range(NWARM):
                tensor.matmul(
                    ps[:64, 0, :64], wt[:, 0, :64], rhs0[:, :64],
                    start=True, stop=True, skip_group_check=True,
                )
            tensor.wait_ge(qa, 16)                  # weights
            for hh in range(2):
                banks = range(4 * hh, 4 * hh + 4)
                tensor.wait_ge(qs, 16 * (hh + 1))   # rhs0 half hh
                for b in banks:
                    bc = slice(b * NCOL_B, (b + 1) * NCOL_B)
                    tensor.matmul(
                        ps[:F, b, :NCOL_B], wt[:, 0, :], rhs0[:, bc],
                        start=True, stop=False,
                    )
                tensor.wait_ge(qa, 16 * (hh + 2))   # wt + rhs1 half hh
                for b in banks:
                    bc = slice(b * NCOL_B, (b + 1) * NCOL_B)
                    tensor.matmul(
                        ps[:F, b, :NCOL_B], wt[:, 1, :], rhs1[:, bc],
                        start=False, stop=True,
                    ).then_inc(msem, 1)

    nc.compile()
    return nc


def _get_nc():
    if "nc" not in _CACHE:
        _CACHE["nc"] = _build_nc()
    return _CACHE["nc"]


def _prepare_in_maps(x, w_real):
    import ml_dtypes

    np_bf = np.dtype(ml_dtypes.bfloat16)
    np_f8 = np.dtype(ml_dtypes.float8_e4m3)
    x = np.ascontiguousarray(np.asarray(x), dtype=np.float32)
    w_real = np.asarray(w_real, dtype=np.float32)

    w2 = w_real[:, 0, KOFF : KOFF + J]                    # [F, J]
    wt = np.empty((K0, 2, F), np.float32)
    wt[:, 0, :] = w2[:, :K0].T
    wt[:, 1, :] = w2[:, K0:].T
    wt = wt.astype(np_bf)

    in_maps = []
    for m in range(NCORES):
        xe = np.zeros((S, XE_LEN), np.float32)
        xe[:, 71 : 71 + J] = x[m * BPC : (m + 1) * BPC].reshape(S, L)[:, :J]
        # interleave: xg[t*S + k] = xe[k, t]
        xet = np.ascontiguousarray(xe.T)                  # [XE_LEN, S]
        xgb = xet.reshape(-1).astype(np_bf)
        xg8 = np.ascontiguousarray(xet[K0:]).reshape(-1).astype(np_f8)
        in_maps.append({"xgb": xgb, "xg8": xg8, "wt": wt})
    return in_maps


def _assemble(results):
    # Device output: y[f, 48*l + k] = out72[f, signal k, l] per core.
    ydev = np.stack([np.asarray(r["y"]) for r in results]).astype(np.float32)
    y = ydev.reshape(NCORES, F, NL, S).transpose(0, 3, 1, 2)  # [8, S, F, NL]
    y = y[..., SEL]                                           # [8, S, F, NI]
    return np.ascontiguousarray(y.reshape(B, C, F, NI))


def kernel(x, w_real):
    nc = _get_nc()
    in_maps = _prepare_in_maps(x, w_real)
    res = run_bass_kernel_spmd(nc, in_maps, list(range(NCORES)))
    return _assemble(res.results)


# revision 7
# speedup vs baseline: 1.0282x; 1.0282x over previous
"""CWTConvNet Trainium2 kernel (v3: raw bacc, 2-pass mixed-precision conv).

The reference computes a 112-filter Morlet-wavelet SAME conv over length-2048
signals, then gathers output positions IMG_SELECT = linspace(0, 71, 224) cast
to int64 — only conv positions 0..71 survive. For those positions, only filter
taps k in [209, 561) touch non-pad input, so the module reduces to

    out72[f, s, l] = sum_{j} w2[f, j] * xe[s, j + l],   l in [0, 72)

with w2 = w_real[:, 0, 209:209+J] and xe = [71 zeros, x[s, :J], ...].

v3 choices (validated numerically; rel err ~6e-3 vs the 2e-2 budget):
- J truncated 352 -> 192: taps >= +121 past the wavelet center carry
  negligible energy. 2 contraction passes of 96 taps each.
- Pass-0 (taps 0..95, incl. the wavelet centers) streams the x-im2col in
  bf16; pass-1 (taps 96..191, Gaussian tails) streams it in fp8-e4m3.
  Weights stay bf16 both passes (mixed-dtype matmul). Output drained
  PSUM->SBUF as bf16. Total DMA ~1.8 MB/core vs 4.1 MB for the baseline.
- Raw bacc with manual semaphores. Both weight passes ride ONE leading DMA
  on the sync ring (tiny-descriptor weight DMAs otherwise starve behind the
  big im2col reads and gate the first matmul). im2col reads are split in
  column halves for matmul/DMA pipelining; per-bank-pair stores chase the
  drains. Dummy warm-up matmuls keep the PE HAM clock-gate busy during the
  DMA fill so the real matmuls run at 2.4 GHz.

Per core (pure data parallel over 4 of 32 batches = 48 signals): one
48-signal element-interleaved group; im2col columns c = 48*l + k so every
DMA row is a 3456 B contiguous run. Host undoes the interleave and applies
the IMG_SELECT repeat-gather on the bf16 result.
"""

import numpy as np

import concourse.bacc as bacc
import concourse.bass as bass
import concourse.mybir as mybir
from concourse.bass_utils import run_bass_kernel_spmd

# Problem constants (hardcoded; kernel.py must be self-contained).
B, C, L = 32, 12, 2048
F = 112
NCORES = 8
BPC = B // NCORES          # batches per core
S = BPC * C                # signals per core (48)
NL = 72                    # conv output positions actually used
NI = 224                   # expanded output length
KOFF = 209                 # first needed tap of the padded-filter window
J = 192                    # taps kept (truncation error ~1e-3 of output norm)
K0 = 96                    # pass-0 taps (bf16)
K1 = J - K0                # pass-1 taps (fp8 x-side), 96
XE_LEN = K0 + K1 - 1 + 71 + 1  # max t touched: 96+95+71 = 262 -> 263
NCOL = S * NL              # 3456 matmul columns
NBANK = 8                  # PSUM banks
NCOL_B = NCOL // NBANK     # 432 columns per bank
QTR = NCOL // 4            # column quarter for pipelining (864)
NWARM_BIG = 4              # 512-col HAM warm-up matmuls (~427 ns each cold)
NWARM_SMALL = 8            # 64-col warm-up matmuls for fine handoff

SEL = np.linspace(0, 71, NI, dtype=np.int64)

f32 = mybir.dt.float32
bf16 = mybir.dt.bfloat16
fp8 = mybir.dt.float8e4

_CACHE = {}


def _build_nc():
    nc = bacc.Bacc("TRN2", target_bir_lowering=False, debug=False)

    xgb_d = nc.declare_dram_parameter("xgb", [XE_LEN * S], bf16, isOutput=False)
    xg8_d = nc.declare_dram_parameter("xg8", [(XE_LEN - K0) * S], fp8, isOutput=False)
    w_d = nc.declare_dram_parameter("wt", [K0, 2, F], bf16, isOutput=False)
    y_d = nc.declare_dram_parameter("y", [F, NCOL], bf16, isOutput=True)

    wt = nc.alloc_sbuf_tensor("wts", [K0, 2, F], bf16)
    rhs0 = nc.alloc_sbuf_tensor("rhs0", [K0, NCOL], bf16)
    rhs1 = nc.alloc_sbuf_tensor("rhs1", [K1, NCOL], fp8)
    o = nc.alloc_sbuf_tensor("o", [F, NCOL], bf16)
    pr0 = nc.alloc_sbuf_tensor("pr0", [1, 8], bf16)
    pr1 = nc.alloc_sbuf_tensor("pr1", [1, 8], bf16)
    ps = nc.alloc_psum_tensor("ps", [128, NBANK, 512], f32)

    # One semaphore per input DMA: a shared counter is racy (16 engine
    # increments can mix across adjacent DMAs on the same ring, firing the
    # threshold before the earlier DMA fully landed).
    qs = [nc.alloc_semaphore(f"qs{q}") for q in range(4)]
    qaw = nc.alloc_semaphore("qaw")
    qa = [nc.alloc_semaphore(f"qa{q}") for q in range(4)]
    msem = nc.alloc_semaphore("msem")  # per-bank matmul-group completions
    vsem = nc.alloc_semaphore("vsem")  # vector drains (even banks)
    ssem = nc.alloc_semaphore("ssem")  # scalar drains (odd banks)
    osem = nc.alloc_semaphore("osem")  # store completions

    def rhs_src(tensor, q):
        return bass.AP(tensor=tensor, offset=QTR * q, ap=[[S, K0], [1, QTR]])

    with nc.Block() as blk:

        @blk.sync
        def _(sync: bass.BassEngine):
            for q in range(4):
                sync.dma_start(
                    rhs0[:, QTR * q : QTR * (q + 1)], rhs_src(xgb_d, q)
                ).then_inc(qs[q], 16)
            # Stores: one per bank pair, chasing the drains.
            for p in range(4):
                sync.wait_ge(vsem, p + 1)
                sync.wait_ge(ssem, p + 1)
                cols = slice(2 * p * NCOL_B, (2 * p + 2) * NCOL_B)
                sync.dma_start(y_d.ap()[:, cols], o[:, cols]).then_inc(osem, 16)
            sync.wait_ge(osem, 64)

        @blk.scalar
        def _(scalar: bass.BassEngine):
            # Weights first: small, and they gate every matmul. On the scalar
            # ring they do not delay the big bf16 reads on the sync ring.
            scalar.dma_start(wt[:], w_d.ap()).then_inc(qaw, 16)
            for q in range(4):
                scalar.dma_start(
                    rhs1[:, QTR * q : QTR * (q + 1)], rhs_src(xg8_d, q)
                ).then_inc(qa[q], 16)
            # Prime the ACT table load during the DMA fill phase, off the
            # drain critical path.
            scalar.copy(pr1[:], pr0[:])
            for b in (1, 3, 5, 7):
                scalar.wait_ge(msem, b + 1)
                cols = slice(b * NCOL_B, (b + 1) * NCOL_B)
                scalar.copy(o[:, cols], ps[:F, b, :NCOL_B]).then_inc(ssem, 1)

        @blk.vector
        def _(vector: bass.BassEngine):
            for b in (0, 2, 4, 6):
                vector.wait_ge(msem, b + 1)
                cols = slice(b * NCOL_B, (b + 1) * NCOL_B)
                vector.tensor_copy(
                    out=o[:, cols], in_=ps[:F, b, :NCOL_B]
                ).then_inc(vsem, 1)

        @blk.tensor
        def _(tensor: bass.BassEngine):
            # HAM warm-up on whatever bytes happen to be in SBUF; results go
            # to a PSUM region every real group later resets (start=True).
            # Long matmuls first (high PE duty so the HAM activity window
            # counts), short ones at the end for a fine handoff to real work.
            for _i in range(NWARM_BIG):
                tensor.matmul(
                    ps[:64, 0, :512], wt[:, 0, :64], rhs0[:, :512],
                    start=True, stop=True, skip_group_check=True,
                )
            for _i in range(NWARM_SMALL):
                tensor.matmul(
                    ps[:64, 0, :64], wt[:, 0, :64], rhs0[:, :64],
                    start=True, stop=True, skip_group_check=True,
                )
            tensor.wait_ge(qaw, 16)                 # weights
            for q in range(4):
                tensor.wait_ge(qs[q], 16)           # rhs0 quarter q
                for b in (2 * q, 2 * q + 1):
                    bc = slice(b * NCOL_B, (b + 1) * NCOL_B)
                    tensor.matmul(
                        ps[:F, b, :NCOL_B], wt[:, 0, :], rhs0[:, bc],
                        start=True, stop=False,
                    )
                tensor.wait_ge(qa[q], 16)           # rhs1 quarter q
                for b in (2 * q, 2 * q + 1):
                    bc = slice(b * NCOL_B, (b + 1) * NCOL_B)
                    tensor.matmul(
                        ps[:F, b, :NCOL_B], wt[:, 1, :], rhs1[:, bc],
                        start=False, stop=True,
                    ).then_inc(msem, 1)

    nc.compile()
    return nc


def _get_nc():
    if "nc" not in _CACHE:
        _CACHE["nc"] = _build_nc()
    return _CACHE["nc"]


def _prepare_in_maps(x, w_real):
    import ml_dtypes

    np_bf = np.dtype(ml_dtypes.bfloat16)
    np_f8 = np.dtype(ml_dtypes.float8_e4m3)
    x = np.ascontiguousarray(np.asarray(x), dtype=np.float32)
    w_real = np.asarray(w_real, dtype=np.float32)

    w2 = w_real[:, 0, KOFF : KOFF + J]                    # [F, J]
    wt = np.empty((K0, 2, F), np.float32)
    wt[:, 0, :] = w2[:, :K0].T
    wt[:, 1, :] = w2[:, K0:].T
    wt = wt.astype(np_bf)

    in_maps = []
    for m in range(NCORES):
        xe = np.zeros((S, XE_LEN), np.float32)
        xe[:, 71 : 71 + J] = x[m * BPC : (m + 1) * BPC].reshape(S, L)[:, :J]
        # interleave: xg[t*S + k] = xe[k, t]
        xet = np.ascontiguousarray(xe.T)                  # [XE_LEN, S]
        xgb = xet.reshape(-1).astype(np_bf)
        xg8 = np.ascontiguousarray(xet[K0:]).reshape(-1).astype(np_f8)
        in_maps.append({"xgb": xgb, "xg8": xg8, "wt": wt})
    return in_maps


def _assemble(results):
    # Device output: y[f, 48*l + k] = out72[f, signal k, l] per core.
    ydev = np.stack([np.asarray(r["y"]) for r in results]).astype(np.float32)
    y = ydev.reshape(NCORES, F, NL, S).transpose(0, 3, 1, 2)  # [8, S, F, NL]
    y = y[..., SEL]                                           # [8, S, F, NI]
    return np.ascontiguousarray(y.reshape(B, C, F, NI))


def kernel(x, w_real):
    nc = _get_nc()
    in_maps = _prepare_in_maps(x, w_real)
    res = run_bass_kernel_spmd(nc, in_maps, list(range(NCORES)))
    return _assemble(res.results)


# revision 8
# speedup vs baseline: 1.0575x; 1.0284x over previous
"""CWTConvNet Trainium2 kernel (v6: raw bacc, 2-pass mixed-precision conv).

The reference computes a 112-filter Morlet-wavelet SAME conv over length-2048
signals, then gathers output positions IMG_SELECT = linspace(0, 71, 224) cast
to int64 — only conv positions 0..71 survive. For those positions, only filter
taps k in [209, 561) touch non-pad input, so the module reduces to

    out72[f, s, l] = sum_{j} w2[f, j] * xe[s, j + l],   l in [0, 72)

with w2 = w_real[:, 0, 209:209+J] and xe = [71 zeros, x[s, :J], ...].

Design (validated numerically; rel err ~5.9e-3 vs the 2e-2 budget):
- J truncated 352 -> 192: taps >= +121 past the wavelet center carry
  negligible energy. 2 contraction passes of 96 taps each.
- Pass-0 (taps 0..95, incl. the wavelet centers) streams the x-im2col in
  bf16; pass-1 (taps 96..191, Gaussian tails) streams it in fp8-e4m3.
  Weights stay bf16 both passes (mixed-dtype matmul). Output drained
  PSUM->SBUF as bf16. Total DMA ~1.8 MB/core vs 4.1 MB for the baseline.
- Raw bacc with manual semaphores, one semaphore per input DMA (a shared
  counter is racy: engine increments mix across adjacent DMAs on a ring).
- The x-im2col streams in pieces (2 single-bank pieces, then quarters) so
  the first matmul starts as early as possible; matmuls are bank-major so
  the PSUM drains (vector/scalar alternating, casting to bf16) and the
  per-bank stores chase the matmul wavefront.
- Dummy warm-up matmuls (long ones for PE-duty, then short ones for a fine
  handoff) keep the PE busy from kernel start straight into the real
  matmuls, so the HAM clock-gate lifts to 2.4 GHz mid-phase.

Per core (pure data parallel over 4 of 32 batches = 48 signals): one
48-signal element-interleaved group; im2col columns c = 48*l + k so every
DMA row is a 3456 B contiguous run. Host undoes the interleave and applies
the IMG_SELECT repeat-gather on the bf16 result.
"""

import numpy as np

import concourse.bacc as bacc
import concourse.bass as bass
import concourse.mybir as mybir
from concourse.bass_utils import run_bass_kernel_spmd

# Problem constants (hardcoded; kernel.py must be self-contained).
B, C, L = 32, 12, 2048
F = 112
NCORES = 8
BPC = B // NCORES          # batches per core
S = BPC * C                # signals per core (48)
NL = 72                    # conv output positions actually used
NI = 224                   # expanded output length
KOFF = 209                 # first needed tap of the padded-filter window
J = 192                    # taps kept (truncation error ~1e-3 of output norm)
K0 = 96                    # pass-0 taps (bf16)
K1 = J - K0                # pass-1 taps (fp8 x-side), 96
XE_LEN = K0 + K1 - 1 + 71 + 1  # max t touched: 96+95+71 = 262 -> 263
NCOL = S * NL              # 3456 matmul columns
NBANK = 8                  # PSUM banks
NCOL_B = NCOL // NBANK     # 432 columns per bank
# Input pieces (in columns): two single-bank pieces then three quarters.
PIECES = (NCOL_B, NCOL_B, 864, 864, 864)
NWARM_BIG = 4              # 512-col HAM warm-up matmuls (~427 ns each cold)
NWARM_SMALL = 24           # 64-col warm-up matmuls for fine handoff

SEL = np.linspace(0, 71, NI, dtype=np.int64)

f32 = mybir.dt.float32
bf16 = mybir.dt.bfloat16
fp8 = mybir.dt.float8e4

_CACHE = {}


def _build_nc():
    nc = bacc.Bacc("TRN2", target_bir_lowering=False, debug=False)

    xgb_d = nc.declare_dram_parameter("xgb", [XE_LEN * S], bf16, isOutput=False)
    xg8_d = nc.declare_dram_parameter("xg8", [(XE_LEN - K0) * S], fp8, isOutput=False)
    w_d = nc.declare_dram_parameter("wt", [K0, 2, F], bf16, isOutput=False)
    y_d = nc.declare_dram_parameter("y", [F, NCOL], bf16, isOutput=True)

    wt = nc.alloc_sbuf_tensor("wts", [K0, 2, F], bf16)
    rhs0 = nc.alloc_sbuf_tensor("rhs0", [K0, NCOL], bf16)
    rhs1 = nc.alloc_sbuf_tensor("rhs1", [K1, NCOL], fp8)
    o = nc.alloc_sbuf_tensor("o", [F, NCOL], bf16)
    pr0 = nc.alloc_sbuf_tensor("pr0", [1, 8], bf16)
    pr1 = nc.alloc_sbuf_tensor("pr1", [1, 8], bf16)
    ps = nc.alloc_psum_tensor("ps", [128, NBANK, 512], f32)

    NP = len(PIECES)
    qs = [nc.alloc_semaphore(f"qs{q}") for q in range(NP)]
    qaw = nc.alloc_semaphore("qaw")
    qa = [nc.alloc_semaphore(f"qa{q}") for q in range(NP)]
    msem = nc.alloc_semaphore("msem")  # per-bank matmul-group completions
    vsem = nc.alloc_semaphore("vsem")  # vector drains (even banks)
    ssem = nc.alloc_semaphore("ssem")  # scalar drains (odd banks)
    osem = nc.alloc_semaphore("osem")  # store completions

    offs = [0]
    for p in PIECES:
        offs.append(offs[-1] + p)
    # banks covered by piece q: [bank_lo[q], bank_lo[q+1])
    bank_lo = [offs[q] // NCOL_B for q in range(NP + 1)]

    def rhs_src(tensor, q):
        return bass.AP(tensor=tensor, offset=offs[q], ap=[[S, K0], [1, PIECES[q]]])

    with nc.Block() as blk:

        @blk.sync
        def _(sync: bass.BassEngine):
            for q in range(NP):
                cols = slice(offs[q], offs[q + 1])
                sync.dma_start(rhs0[:, cols], rhs_src(xgb_d, q)).then_inc(qs[q], 16)
            # Stores: one per bank, chasing the drains.
            for b in range(NBANK):
                sync.wait_ge(vsem if b % 2 == 0 else ssem, b // 2 + 1)
                cols = slice(b * NCOL_B, (b + 1) * NCOL_B)
                sync.dma_start(y_d.ap()[:, cols], o[:, cols]).then_inc(osem, 16)
            sync.wait_ge(osem, 16 * NBANK)

        @blk.scalar
        def _(scalar: bass.BassEngine):
            # Weights first: small, and they gate every matmul. On the scalar
            # ring they do not delay the big bf16 reads on the sync ring.
            scalar.dma_start(wt[:], w_d.ap()).then_inc(qaw, 16)
            for q in range(NP):
                cols = slice(offs[q], offs[q + 1])
                scalar.dma_start(rhs1[:, cols], rhs_src(xg8_d, q)).then_inc(qa[q], 16)
            # Prime the ACT table load during the DMA fill phase, off the
            # drain critical path.
            scalar.copy(pr1[:], pr0[:])
            for b in (1, 3, 5, 7):
                scalar.wait_ge(msem, b + 1)
                cols = slice(b * NCOL_B, (b + 1) * NCOL_B)
                scalar.copy(o[:, cols], ps[:F, b, :NCOL_B]).then_inc(ssem, 1)

        @blk.vector
        def _(vector: bass.BassEngine):
            for b in (0, 2, 4, 6):
                vector.wait_ge(msem, b + 1)
                cols = slice(b * NCOL_B, (b + 1) * NCOL_B)
                vector.tensor_copy(
                    out=o[:, cols], in_=ps[:F, b, :NCOL_B]
                ).then_inc(vsem, 1)

        @blk.tensor
        def _(tensor: bass.BassEngine):
            # HAM warm-up on whatever bytes happen to be in SBUF; results go
            # to a PSUM region every real group later resets (start=True).
            # Long matmuls first (high PE duty so the HAM activity window
            # counts), short ones at the end so the handoff to the real
            # matmuls leaves no idle gap that would reset the HAM window.
            for _i in range(NWARM_BIG):
                tensor.matmul(
                    ps[:64, 0, :512], wt[:, 0, :64], rhs0[:, :512],
                    start=True, stop=True, skip_group_check=True,
                )
            for _i in range(NWARM_SMALL):
                tensor.matmul(
                    ps[:64, 0, :64], wt[:, 0, :64], rhs0[:, :64],
                    start=True, stop=True, skip_group_check=True,
                )
            tensor.wait_ge(qaw, 16)                 # weights
            for q in range(NP):
                tensor.wait_ge(qs[q], 16)           # rhs0 piece q
                for b in range(bank_lo[q], bank_lo[q + 1]):
                    bc = slice(b * NCOL_B, (b + 1) * NCOL_B)
                    tensor.matmul(
                        ps[:F, b, :NCOL_B], wt[:, 0, :], rhs0[:, bc],
                        start=True, stop=False,
                    )
                tensor.wait_ge(qa[q], 16)           # rhs1 piece q
                for b in range(bank_lo[q], bank_lo[q + 1]):
                    bc = slice(b * NCOL_B, (b + 1) * NCOL_B)
                    tensor.matmul(
                        ps[:F, b, :NCOL_B], wt[:, 1, :], rhs1[:, bc],
                        start=False, stop=True,
                    ).then_inc(msem, 1)

    nc.compile()
    return nc


def _get_nc():
    if "nc" not in _CACHE:
        _CACHE["nc"] = _build_nc()
    return _CACHE["nc"]


def _prepare_in_maps(x, w_real):
    import ml_dtypes

    np_bf = np.dtype(ml_dtypes.bfloat16)
    np_f8 = np.dtype(ml_dtypes.float8_e4m3)
    x = np.ascontiguousarray(np.asarray(x), dtype=np.float32)
    w_real = np.asarray(w_real, dtype=np.float32)

    w2 = w_real[:, 0, KOFF : KOFF + J]                    # [F, J]
    wt = np.empty((K0, 2, F), np.float32)
    wt[:, 0, :] = w2[:, :K0].T
    wt[:, 1, :] = w2[:, K0:].T
    wt = wt.astype(np_bf)

    in_maps = []
    for m in range(NCORES):
        xe = np.zeros((S, XE_LEN), np.float32)
        xe[:, 71 : 71 + J] = x[m * BPC : (m + 1) * BPC].reshape(S, L)[:, :J]
        # interleave: xg[t*S + k] = xe[k, t]
        xet = np.ascontiguousarray(xe.T)                  # [XE_LEN, S]
        xgb = xet.reshape(-1).astype(np_bf)
        xg8 = np.ascontiguousarray(xet[K0:]).reshape(-1).astype(np_f8)
        in_maps.append({"xgb": xgb, "xg8": xg8, "wt": wt})
    return in_maps


def _assemble(results):
    # Device output: y[f, 48*l + k] = out72[f, signal k, l] per core.
    ydev = np.stack([np.asarray(r["y"]) for r in results]).astype(np.float32)
    y = ydev.reshape(NCORES, F, NL, S).transpose(0, 3, 1, 2)  # [8, S, F, NL]
    y = y[..., SEL]                                           # [8, S, F, NI]
    return np.ascontiguousarray(y.reshape(B, C, F, NI))


def kernel(x, w_real):
    nc = _get_nc()
    in_maps = _prepare_in_maps(x, w_real)
    res = run_bass_kernel_spmd(nc, in_maps, list(range(NCORES)))
    return _assemble(res.results)
